# revision 1
# baseline (speedup 1.0000x reference)
"""YOLO-style detection decode (nms_detection) on 8 trn2 NeuronCores.

Data-parallel over batch (64 -> 8 images/core). All per-core inputs are
packed into ONE flat f32 DRAM tensor (x52|x26|x13 in natural [b,ch,s]
order, then small constants) and the result is ONE [28392, 18] f32
tensor (cells x (3 anchors x 6)), reassembled on the host. One input +
one output minimizes the large per-tensor dispatch overhead of the
execution path.

Device pipeline per 4-chunk group (chunk = 128 cells):
  - segment DMAs load [128ch, cells] strips (raw channel order).
  - PE transposes chunks into PSUM -> [cell, 255ch].
  - DVE reduce_max over the 80 class cols per anchor -> m (exact).
  - PE transposes m; an fp32 K=3 matmul subtracts m from the class
    logits (exact: Sterbenz near the max) and a K=1 matmul adds
    (79-c)*2^-31. The winner's value is then exactly
    (79-argmax)*2^-31 >= 0 while every loser stays < 0, so a second
    DVE reduce_max recovers argmax exactly (incl. first-index ties,
    matching jnp.argmax).
  - decode: conf = sigmoid (ACT), cx/cy fused scalar_tensor_tensor with
    host grid offsets, w/h = exp * anchors/416, mask = (logit > 0)
    applied multiplicatively (fused is_gt*mult per anchor).
"""

import os
from contextlib import ExitStack

import numpy as np

import concourse.bass as bass
import concourse.tile as tile
from concourse import bacc, mybir
from concourse.bass_utils import run_bass_kernel_spmd

N_CORES = 8
B = 64
B_PER = B // N_CORES
CASE = 416.0
SCALES = [("52", 52, 8.0), ("26", 26, 16.0), ("13", 13, 32.0)]
CHUNK = 128
GRP = 4
F32 = mybir.dt.float32
AX = mybir.AxisListType
OP = mybir.AluOpType
AF = mybir.ActivationFunctionType
IOTA_SCALE = 2.0 ** -31


def _cells(h):
    return B_PER * h * h


def _nchunks(h):
    return (_cells(h) + CHUNK - 1) // CHUNK


def _gxy_section(h, t):
    n = _cells(h)
    nch = _nchunks(h)
    cells = np.arange(nch * CHUNK)
    s = cells % (h * h)
    gx = (s % h).astype(np.float64) * t / CASE
    gy = (s // h).astype(np.float64) * t / CASE
    gx[cells >= n] = 0.0
    gy[cells >= n] = 0.0
    out = np.zeros((CHUNK, 2 * nch), np.float32)
    for j in range(nch):
        out[:, 2 * j] = gx[j * CHUNK:(j + 1) * CHUNK]
        out[:, 2 * j + 1] = gy[j * CHUNK:(j + 1) * CHUNK]
    return out


def _consts():
    import ml_dtypes
    bf = ml_dtypes.bfloat16
    # raw channel order: anchor a's class cols at 85a+5 .. 85a+85.
    # sel9 rows 32q + (3*term + a): -1 selector for the 3-term bf16 split.
    sel9 = np.zeros((128, 256), bf)
    for q in range(4):
        for r in range(9):
            a = r % 3
            sel9[32 * q + r, 85 * a + 5:85 * a + 85] = -1.0
    iotam = np.zeros((1, 256), bf)
    for a in range(3):
        iotam[0, 85 * a + 5:85 * a + 85] = \
            ((79.0 - np.arange(80)) * IOTA_SCALE).astype(bf)
    onesb = np.ones((1, 128), bf)
    iden = np.eye(128, dtype=np.float32)
    gxy = np.concatenate([_gxy_section(h, t) for _, h, t in SCALES], axis=1)
    return {
        "gxy": gxy.astype(np.float32),
        "iden": iden,
        "sel9": sel9.view(np.float32),
        "iotam": iotam.view(np.float32),
        "onesb": onesb.view(np.float32),
    }


_CONSTS = _consts()

# packed input layout (f32 elements, per core)
_X_OFF = {}
_off = 0
for _tag, _h, _t in SCALES:
    _X_OFF[_tag] = _off
    _off += B_PER * 255 * _h * _h
_CONST_OFF = {}
for _name in ("gxy", "iden", "sel9", "iotam", "onesb"):
    _CONST_OFF[_name] = _off
    _off += _CONSTS[_name].size
_CONST_OFF["anch"] = _off
_off += 128 * 18
TOTAL_IN = _off

_O_OFF = {}
_off = 0
for _tag, _h, _t in SCALES:
    _O_OFF[_tag] = _off
    _off += _cells(_h)
TOTAL_OUT_ROWS = _off  # 28392


def _a85(ap_pgx, lo, width=1):
    """[128, gc, 3(anchor), width] view of box channel `lo` from a
    [128, gc, 512] psum group view (channel stride 85)."""
    v = ap_pgx[:, :, 0:255].rearrange("p g (a r) -> p g a r", a=3, r=85)
    return v[:, :, :, lo:lo + width]


def _emit_scale(nc, tc, ctx, pools, sb, xin, oX, h, t, tag, gxy_off):
    ST = int(os.environ.get("KSTAGE", "9"))
    n = _cells(h)
    hw = h * h
    nch = _nchunks(h)
    ngrp = (nch + GRP - 1) // GRP
    k = float(t / CASE)
    (p_ina, p_inb, p_ps, p_m, p_mt, p_out) = pools

    xoff = _X_OFF[tag]
    xr3 = xin[xoff:xoff + B_PER * 255 * hw] \
        .rearrange("(b c s) -> c b s", b=B_PER, c=255)

    def seg_dma(dst_tile, nrows, src0, c0, w):
        done = 0
        while done < w:
            cell = c0 + done
            b = cell // hw
            s = cell % hw
            span = min(w - done, hw - s)
            nc.sync.dma_start(dst_tile[0:nrows, done:done + span],
                              xr3[src0:src0 + nrows, b, s:s + span])
            done += span

    for g in range(ngrp):
        j0 = g * GRP
        gc = min(GRP, nch - j0)
        c0 = j0 * CHUNK
        w = min(GRP * CHUNK, n - c0)

        in_a = p_ina.tile([128, GRP * CHUNK], F32, tag="in_a")
        in_b = p_inb.tile([128, GRP * CHUNK], F32, tag="in_b")
        seg_dma(in_a, 128, 0, c0, w)
        seg_dma(in_b, 127, 128, c0, w)

        ps = p_ps.tile([128, 4 * 512], F32, tag="ps")
        pg = ps[:].rearrange("p (g x) -> p g x", g=4)[:, 0:gc, :]
        ncs = []
        for jj in range(gc):
            ncj = min(CHUNK, w - jj * CHUNK)
            ncs.append(ncj)
            if ncj < CHUNK:
                nc.vector.memset(ps[:, jj * 512:jj * 512 + 255], 0.0)
            nc.tensor.transpose(ps[0:ncj, jj * 512:jj * 512 + 128],
                                in_a[:, jj * CHUNK:jj * CHUNK + ncj],
                                sb["iden"])
            nc.tensor.matmul(ps[0:ncj, jj * 512 + 128:jj * 512 + 255],
                             in_b[0:127, jj * CHUNK:jj * CHUNK + ncj],
                             sb["iden"][0:127, 0:127],
                             is_transpose=True, start=False, stop=True,
                             skip_group_check=True)

        cls_ap = _a85(pg, 5, 80)          # [128, gc, 3, 80]
        conf_ap = _a85(pg, 0).squeeze(3)  # [128, gc, 3]

        # ---- scan 1: exact class max ----
        m_sb = p_m.tile([128, 12], F32, tag="m_sb")
        m_v = m_sb[:].rearrange("p (g a) -> p g a", g=4)[:, 0:gc, :]
        if ST >= 2:
            nc.vector.tensor_reduce(m_v, cls_ap, axis=AX.X, op=OP.max)
        else:
            nc.vector.memset(m_sb[:, :], 0.0)

        # ---- exact 3-term bf16 split of m (gpsimd, off critical engines):
        # m = h1 + h2 + h3 with every term bf16-representable.
        BF16 = mybir.dt.bfloat16
        hb = p_m.tile([128, 12], BF16, tag="hb")
        hb2 = p_m.tile([128, 12], BF16, tag="hb2")
        r1 = p_m.tile([128, 12], F32, tag="r1")
        msp = p_m.tile([128, 128], F32, tag="msp")
        hb_v = hb[:].rearrange("p (g a) -> p g a", g=4)[:, 0:gc, :]
        hb2_v = hb2[:].rearrange("p (g a) -> p g a", g=4)[:, 0:gc, :]
        r1_v = r1[:].rearrange("p (g a) -> p g a", g=4)[:, 0:gc, :]
        mspv = msp[:].rearrange("p (g r) -> p g r", g=4)
        if ST >= 3:
            nc.vector.memset(msp[:, :], 0.0)
            nc.vector.tensor_copy(hb_v, m_v)
            nc.vector.tensor_copy(mspv[:, 0:gc, 0:3], hb_v)
            nc.vector.tensor_tensor(r1_v, m_v, hb_v, op=OP.subtract)
            nc.vector.tensor_copy(hb2_v, r1_v)
            nc.vector.tensor_copy(mspv[:, 0:gc, 3:6], hb2_v)
            nc.vector.tensor_tensor(mspv[:, 0:gc, 6:9], r1_v, hb2_v,
                                    op=OP.subtract)

        # ---- transpose m-split into psum spare (halves: bases 0/32) ----
        mts = []
        for hh in range((gc + 1) // 2 if ST >= 4 else 0):
            nc.tensor.matmul(ps[0:64, hh * 512 + 256:hh * 512 + 384],
                             msp[:, 64 * hh:64 * hh + 64],
                             sb["iden"][0:128, 0:128],
                             is_transpose=True, start=False, stop=True,
                             skip_group_check=True)
            mt_t = p_mt.tile([64, 128], BF16, tag=f"mtsb{hh}")
            nc.scalar.copy(mt_t[:, :],
                           ps[0:64, hh * 512 + 256:hh * 512 + 384])
            mts.append(mt_t)

        # ---- recenter: cls += -m, then += iota (separate accumulates) --
        for jj in range(gc if ST >= 5 else 0):
            out_cls = ps[:, jj * 512:jj * 512 + 255]
            bp = 32 * (jj % 2)
            nc.tensor.matmul(out_cls, mts[jj // 2][bp:bp + 9, :],
                             sb["sel9"][bp:bp + 9, 0:255],
                             start=False, stop=True, skip_group_check=True)
            nc.tensor.matmul(out_cls, sb["onesb"], sb["iotam"][:, 0:255],
                             start=False, stop=True, skip_group_check=True)

        # ---- scan 2: argmax ----
        idx_sb = p_m.tile([128, 12], F32, tag="idx_sb")
        idx_v = idx_sb[:].rearrange("p (g a) -> p g a", g=4)[:, 0:gc, :]
        if ST >= 6:
            nc.vector.tensor_reduce(idx_v, cls_ap, axis=AX.X, op=OP.max)
        else:
            nc.vector.memset(idx_sb[:, :], 0.0)

        # ---- decode ----
        out4 = p_out.tile([128, GRP * 18], F32, tag="out4")
        if ST < 7:
            nc.vector.memset(out4[:, :], 0.0)
        o4 = out4[:].rearrange("p (g a s) -> p g a s", g=4, a=3)
        o4t = out4[:].rearrange("p (g a s) -> p g s a", g=4, a=3)

        if ST >= 7:
            # conf = 1/(1 + exp(-logit)): stay in the Exp table set
            econf = p_m.tile([128, 12], F32, tag="econf")
            e_v = econf[:].rearrange("p (g a) -> p g a", g=4)[:, 0:gc, :]
            nc.scalar.activation(e_v, conf_ap, AF.Exp, scale=-1.0)
            ep1 = p_m.tile([128, 12], F32, tag="ep1")
            e1_v = ep1[:].rearrange("p (g a) -> p g a", g=4)[:, 0:gc, :]
            nc.vector.tensor_scalar(e1_v, e_v, 1.0, None, op0=OP.add)
            nc.vector.reciprocal(o4t[:, 0:gc, 0:1, :].squeeze(2), e1_v)

            gxy_ap = sb["gxy"][:, gxy_off + 2 * j0:gxy_off + 2 * j0 + 2 * gc]
            gxy_r = gxy_ap.rearrange("p (g q) -> p g q", q=2)
            for kk in range(2):
                g_v = gxy_r[:, :, kk:kk + 1].broadcast_to([128, gc, 3])
                src = _a85(pg, 1 + kk).squeeze(3)
                dst = o4t[:, 0:gc, 1 + kk:2 + kk, :].squeeze(2)
                nc.vector.scalar_tensor_tensor(dst, src, k, g_v,
                                               op0=OP.mult, op1=OP.add)

            twh = p_m.tile([128, 24], F32, tag="twh")
            twh_v = twh[:].rearrange("p (g q a) -> p g q a", g=4, q=2)
            for kk in range(2):
                nc.scalar.activation(
                    twh_v[:, 0:gc, kk:kk + 1, :].squeeze(2),
                    _a85(pg, 3 + kk).squeeze(3), AF.Exp)
            anch_v = sb["anch"].rearrange("p (q a) -> p q a", q=2) \
                .unsqueeze(1).broadcast_to([128, gc, 2, 3])
            nc.vector.tensor_tensor(o4t[:, 0:gc, 3:5, :],
                                    twh_v[:, 0:gc], anch_v, op=OP.mult)

            nc.scalar.activation(o4t[:, 0:gc, 5:6, :].squeeze(2), idx_v,
                                 AF.Copy, bias=79.0, scale=-(2.0 ** 31))

            for a in range(3):
                cb = conf_ap[:, :, a:a + 1].broadcast_to([128, gc, 6])
                dst = o4[:, 0:gc, a, :]
                nc.vector.scalar_tensor_tensor(dst, cb, 0.0, dst,
                                               op0=OP.is_gt, op1=OP.mult)

        nfull = sum(1 for x in ncs if x == CHUNK)
        r0 = _O_OFF[tag] + c0
        if nfull:
            dst = oX[r0:r0 + nfull * CHUNK, :] \
                .rearrange("(g p) c -> p g c", p=CHUNK)
            nc.sync.dma_start(dst, o4[:, 0:nfull].rearrange(
                "p g a s -> p g (a s)"))
        if nfull < gc:
            ncj = ncs[nfull]
            rp = r0 + nfull * CHUNK
            nc.sync.dma_start(oX[rp:rp + ncj, :],
                              out4[0:ncj, 18 * nfull:18 * nfull + 18])


def build():
    nc = bacc.Bacc("TRN2", target_bir_lowering=False, debug=False,
                   num_devices=N_CORES)
    xin = nc.dram_tensor("xin", [TOTAL_IN], F32, kind="ExternalInput").ap()
    oX = nc.dram_tensor("out", [TOTAL_OUT_ROWS, 18], F32,
                        kind="ExternalOutput").ap()

    with tile.TileContext(nc) as tc:
        with ExitStack() as ctx:
            p_c = ctx.enter_context(tc.tile_pool(name="consts", bufs=1))
            p_ina = ctx.enter_context(tc.tile_pool(name="inpa", bufs=4))
            p_inb = ctx.enter_context(tc.tile_pool(name="inpb", bufs=4))
            p_ps = ctx.enter_context(
                tc.tile_pool(name="ps", bufs=2, space="PSUM"))
            p_m = ctx.enter_context(tc.tile_pool(name="small", bufs=3))
            p_mt = ctx.enter_context(tc.tile_pool(name="mt", bufs=3))
            p_out = ctx.enter_context(tc.tile_pool(name="out", bufs=4))

            shapes = {"gxy": [128, _CONSTS["gxy"].shape[1]],
                      "iden": [128, 128], "sel9": [128, 128],
                      "iotam": [1, 128], "onesb": [1, 64],
                      "anch": [128, 18]}
            sb = {}
            for name, shp in shapes.items():
                t_ = p_c.tile(shp, F32, tag=name)
                size = shp[0] * shp[1]
                src = xin[_CONST_OFF[name]:_CONST_OFF[name] + size] \
                    .rearrange("(p f) -> p f", p=shp[0])
                nc.sync.dma_start(t_[:], src)
                if name in ("sel9", "iotam", "onesb"):
                    sb[name] = t_[:].bitcast(mybir.dt.bfloat16)
                else:
                    sb[name] = t_[:]
            anch_t = sb["anch"]

            pools = (p_ina, p_inb, p_ps, p_m, p_mt, p_out)
            for _rep in range(int(os.environ.get("KREP", "1"))):
                gxy_off = 0
                anch_off = 0
                for tag, h, t in SCALES:
                    sbs = dict(sb)
                    sbs["anch"] = anch_t[:, anch_off:anch_off + 6]
                    _emit_scale(nc, tc, ctx, pools, sbs, xin, oX, h, t,
                                tag, gxy_off)
                    gxy_off += 2 * _nchunks(h)
                    anch_off += 6
    nc.compile()
    return nc


_NC = None


def _get_nc():
    global _NC
    if _NC is None:
        _NC = build()
    return _NC


def _make_anch(anchors):
    anch = np.zeros((128, 18), np.float32)
    off = 0
    for tag, h, _ in SCALES:
        a = anchors[tag].astype(np.float64) / CASE
        for kk in range(2):
            for aa in range(3):
                anch[:, off + kk * 3 + aa] = a[aa, kk]
        off += 6
    return anch


def _pack_core(xs, anch):
    parts = [np.asarray(xs["52"]).ravel(), np.asarray(xs["26"]).ravel(),
             np.asarray(xs["13"]).ravel(),
             _CONSTS["gxy"].ravel(), _CONSTS["iden"].ravel(),
             _CONSTS["sel9"].ravel(), _CONSTS["iotam"].ravel(),
             _CONSTS["onesb"].ravel(), anch.ravel()]
    out = np.concatenate(parts)
    assert out.size == TOTAL_IN and out.dtype == np.float32
    return out


def kernel(out13, out26, out52, anchors13, anchors26, anchors52):
    nc = _get_nc()
    xs_all = {"13": np.asarray(out13), "26": np.asarray(out26),
              "52": np.asarray(out52)}
    anchors = {"13": np.asarray(anchors13), "26": np.asarray(anchors26),
               "52": np.asarray(anchors52)}
    anch = _make_anch(anchors)

    in_maps = []
    for i in range(N_CORES):
        xs = {tag: xs_all[tag][i * B_PER:(i + 1) * B_PER]
              for tag, _, _ in SCALES}
        in_maps.append({"xin": _pack_core(xs, anch)})

    res = run_bass_kernel_spmd(nc, in_maps, list(range(N_CORES))).results

    parts = []
    for tag, h, _ in SCALES[::-1]:  # output order: 13, 26, 52
        o0 = _O_OFF[tag]
        for i in range(N_CORES):
            parts.append(res[i]["out"][o0:o0 + _cells(h)].reshape(-1, 6))
    return np.concatenate(parts, axis=0)

